# revision 20
# baseline (speedup 1.0000x reference)
"""Trainium2 Bass kernel for Conv2D_DT (distance-transform conv).

d(n,o,h,w) = || patch(n,:,h,w) - W[o,:] ||_2  with 3x3/pad1 im2col patches.

Strategy (8 NeuronCores, data-parallel over batch; fp8 DoubleRow taps):
  - 4 images per core as 2 pairs: image A on SBUF partitions 0-63,
    image B on 64-127 (channels = partition dim). K=64 matmuls for the
    two images land on PE row-groups (0,0)/(64,0) and run concurrently.
  - x is shipped ONCE as fp8e4 (value 8*x), padded to [C, 58, 64] so an
    8-row x 64-col chunk is contiguous (512 f32 = exactly one PSUM bank)
    and DoubleRow k-tile strides are 16B-aligned.
  - d2 = ||p||^2 + ||w||^2 - 2 p.w in PSUM, scale 256 = (8)*(32):
      * 6 fp8 DoubleRow matmuls per chunk: row pairs (0,kw)+(1,kw) and
        (2,kw)+zero-phantom, lhsT = fp8(-64*W_tap): 2 taps per pass.
      * 1 bf16 matmul with lhsT = 4.0 over b = 3x3 box sum of (8x)^2,
        computing the whole 256*||p||^2 term via the K=64 contraction.
  - box sums run as bf16 scalar_tensor_tensor on flat [128, R*64] views
    (packed 2-byte operands -> 4x DVE mode); squares (fp8 in, 1x) are
    split between GpSimd and DVE to keep both below the DMA roofline.
  - epilogue: ScalarE  out = Sqrt(psum/256 + w2[o]), two chunks batched
    into one [128,16,56] tile per image -> 3.5KB/partition output DMAs.
  - chunk b-matmul/epilogue deferred 3 chunks (8 PSUM banks) so the PE
    queue front is taps only; preprocessing never stalls the PE.
"""

import sys

_REPO = "/opt/trn_rl_repo"
if _REPO not in sys.path:
    sys.path.insert(0, _REPO)

import bass_rust
import ml_dtypes
import numpy as np

import concourse.bass as bass  # noqa: F401
import concourse.mybir as mybir
import concourse.tile as tile
from concourse import bacc
from concourse.bass_utils import run_bass_kernel_spmd

# Problem geometry (hardcoded per harness contract).
N, C, H, W_DIM, O = 32, 64, 56, 56, 128
NCORES = 8
NL = N // NCORES  # images per core
NPAIR = NL // 2  # image pairs per core
HP = 58  # zero-padded row count
WP = 64  # padded row width (56 data + 1+1 zero pad + 6 zero tail)
RCH = 8  # output rows per PSUM chunk ([128, 8, 64] f32 = one bank)
NCH = H // RCH  # 7 chunks per image
NGRP = 6  # DoubleRow tap groups per chunk
DELAY = 3  # chunks between taps and b-matmul/epilogue (8 PSUM banks)
XS, WS = 8.0, 32.0  # fp8 pre-scales for x and -2W (epilogue undoes 1/256)
SWI = True  # DoubleRowSwInterleave: software-interleaved weights (FWL-able)

F32 = mybir.dt.float32
BF16 = mybir.dt.bfloat16
FP8 = mybir.dt.float8e4

_PROGRAM = None


def _flat(t, row, col, count):
    """[128, count] stride-1 view of tile t starting at (row, col)."""
    sl = t[0:128, row : row + 1, col : col + 1]
    sl.ap = bass_rust.VecI64Pair([list(sl.ap[0]), [1, count]])
    return sl


def _dr_rhs(t, hsl, row, col):
    """DoubleRow ifmap view [64, 2, 8, 64]: k-tile stride WP (next row),
    8 output rows x 64 cols contiguous."""
    sl = t[hsl, row : row + 1, col : col + 1]
    sl.ap = bass_rust.VecI64Pair([list(sl.ap[0]), [WP, 2], [WP, RCH], [1, WP]])
    return sl


def _build_program():
    nc = bacc.Bacc(
        "TRN2",
        target_bir_lowering=False,
        debug=False,
        enable_asserts=False,
        num_devices=NCORES,
    )
    xs8 = nc.dram_tensor("xs8", [NL, C, HP, WP], FP8, kind="ExternalInput")
    lwf = nc.dram_tensor("lwf", [128, NGRP, 2, 128], FP8, kind="ExternalInput")
    lwo = nc.dram_tensor("lwo", [128, 128], BF16, kind="ExternalInput")
    w2 = nc.dram_tensor("w2", [128, 1], F32, kind="ExternalInput")
    out = nc.dram_tensor("out", [NL, O, H, W_DIM], F32, kind="ExternalOutput")

    DR = (
        mybir.MatmulPerfMode.DoubleRowSwInterleave
        if SWI
        else mybir.MatmulPerfMode.DoubleRow
    )
    MUL = mybir.AluOpType.mult
    ADD = mybir.AluOpType.add

    with tile.TileContext(nc) as tc:
        with (
            tc.tile_pool(name="const", bufs=1) as cpool,
            tc.tile_pool(name="xin", bufs=4) as xpool,
            tc.tile_pool(name="imgs", bufs=2) as ipool,
            tc.tile_pool(name="outs", bufs=4) as opool,
            tc.tile_pool(name="psum", bufs=8, space="PSUM") as ppool,
        ):
            lwft = cpool.tile([128, NGRP, 2, 128], FP8)
            nc.sync.dma_start(out=lwft[:], in_=lwf[:, :, :, :])
            lwot = cpool.tile([128, 128], BF16)
            nc.sync.dma_start(out=lwot[:], in_=lwo[:, :])
            w2t = cpool.tile([128, 1], F32)
            nc.sync.dma_start(out=w2t[:], in_=w2[:, :])

            # halves: (first padded row, DMA'd rows, chunks); tiles get two
            # extra zeroed rows so shifted/wrapping views never leave the
            # tile or read stale SBUF.
            HALVES = ((0, 35, (0, 1, 2, 3)), (32, 26, (4, 5, 6)))

            # Front-load every input DMA (and spare-row memsets) so the DMA
            # queue head is pure input while compute ramps.
            pair_x = []
            for p in range(NPAIR):
                na, nb = 2 * p, 2 * p + 1
                xh = []
                for r0, RD, _chs in HALVES:
                    x8t = xpool.tile([128, RD + 2, WP], FP8, tag="x8")
                    # one descriptor for both images: partition = img*64+ch;
                    # issued on the ACT hwdge queue so descriptor generation
                    # overlaps the sync queue's const/output DMAs.
                    nc.scalar.dma_start(
                        out=x8t[:, 0:RD, :],
                        in_=xs8[na : nb + 1, :, r0 : r0 + RD, :],
                    )
                    nc.vector.memset(x8t[:, RD : RD + 2, :], 0)
                    xh.append(x8t)
                pair_x.append(xh)

            # ot2 assembly: two chunks' sqrt outputs share one DMA tile.
            ot2_cur = {}

            def finish(item):
                ch, na, nb, psa, psb, bt, r0 = item
                lb = ch * RCH - r0
                for hsl, ps in ((slice(0, 64), psa), (slice(64, 128), psb)):
                    for kh in range(3):  # vertical 3x1 box sum on the PE
                        nc.tensor.matmul(
                            ps[:],
                            lwot[hsl, :],
                            bt[hsl, lb + kh : lb + kh + RCH, :],
                            start=False,
                            stop=kh == 2,
                        )
                slot = ch % 2  # two chunks share one output tile
                hh = 8 if ch == 6 else 16
                for ps, n_img in ((psa, na), (psb, nb)):
                    if slot == 0:
                        ot2_cur[n_img] = opool.tile(
                            [128, 16, W_DIM], F32, tag="ot", name="ot"
                        )
                    ot = ot2_cur[n_img]
                    nc.scalar.activation(
                        out=ot[:, slot * RCH : slot * RCH + RCH, :],
                        in_=ps[:, :, 0:W_DIM],
                        func=mybir.ActivationFunctionType.Sqrt,
                        bias=w2t[:],
                        scale=1.0 / 256.0,
                    )
                    if slot == 1 or ch == 6:
                        h0 = (ch - slot) * RCH
                        nc.sync.dma_start(
                            out=out[n_img, :, h0 : h0 + hh, :], in_=ot[:, 0:hh, :]
                        )

            pending = []
            for p in range(NPAIR):
                na, nb = 2 * p, 2 * p + 1
                halves = []
                for (r0, RD, chs), x8t in zip(HALVES, pair_x[p]):
                    nr = RD + 1  # sq rows (incl one zeroed spare)
                    sqt = ipool.tile([128, nr, WP], BF16, tag="sq")
                    sqs = ipool.tile([128, RD, WP], BF16, tag="sqs")
                    ut = ipool.tile([128, RD, WP], BF16, tag="u")
                    ttt = ipool.tile([128, RD, WP], BF16, tag="tt")
                    # The very first half gates the PE pipeline: emit a short
                    # leading row-slice first so chunk 0's box term is ready
                    # a few us earlier, then the remainder.
                    # ((square rows), (shifted-op rows)) per segment; shifted
                    # ops lag the square by a row (they peek into row+1).
                    if p == 0 and r0 == 0:
                        segs = (((0, 12), (0, 11)), ((12, nr), (11, RD)))
                    else:
                        segs = (((0, nr), (0, RD)),)
                    for (q0, q1), (s0, s1) in segs:
                        ns = (s1 - s0) * WP  # rows for shifted views
                        nf = (q1 - q0) * WP  # rows for the square
                        # squares (fp8 in, 1x anywhere): first half on ScalarE
                        # (its sqrt backlog is empty early), rest on GpSimd.
                        if r0 == 0:
                            nc.scalar.activation(
                                out=_flat(sqt, q0, 0, nf),
                                in_=_flat(x8t, q0, 0, nf),
                                func=mybir.ActivationFunctionType.Square,
                            )
                        else:
                            nc.gpsimd.tensor_mul(
                                _flat(sqt, q0, 0, nf),
                                _flat(x8t, q0, 0, nf),
                                _flat(x8t, q0, 0, nf),
                            )
                        # +1-elem shifted replica via single-src copy (2x_2p
                        # needs no alignment); keeps adds 4B-aligned for 2x_1p.
                        nc.vector.tensor_copy(
                            _flat(sqs, s0, 0, ns), _flat(sqt, s0, 1, ns)
                        )
                        nc.vector.tensor_add(
                            _flat(ut, s0, 0, ns),
                            _flat(sqt, s0, 0, ns),
                            _flat(sqs, s0, 0, ns),
                        )
                        nc.vector.tensor_add(
                            _flat(ttt, s0, 0, ns),
                            _flat(ut, s0, 0, ns),
                            _flat(sqt, s0, 2, ns),
                        )
                    halves.append((r0, chs, x8t, ttt))

                for r0, chs, x8t, bt in halves:
                    for ch in chs:
                        lh = ch * RCH - r0  # chunk's first row, local to half
                        psa = ppool.tile([128, RCH, WP], F32, tag="ps")
                        psb = ppool.tile([128, RCH, WP], F32, tag="ps")
                        for g in range(NGRP):
                            kh0, kw = (0, g) if g < 3 else (2, g - 3)
                            st = g == 0
                            for hsl, ps in ((slice(0, 64), psa), (slice(64, 128), psb)):
                                nc.tensor.matmul(
                                    ps[:],
                                    lwft[hsl, g],
                                    _dr_rhs(x8t, hsl, lh + kh0, kw),
                                    start=st,
                                    stop=False,
                                    perf_mode=DR,
                                )
                        pending.append((ch, na, nb, psa, psb, bt, r0))
                        if len(pending) > DELAY:
                            finish(pending.pop(0))
            for item in pending:
                finish(item)
    nc.compile()
    return nc


def _host_weights(W):
    """fp8 DoubleRow lhsT [128, 6, 2, 128], bf16 4.0 matrix, f32 w2."""
    W = np.asarray(W, np.float32)
    cidx = np.arange(C)
    lwf = np.zeros((128, NGRP, 2, 128), np.float32)
    for g in range(NGRP):
        kh0, kw = (0, g) if g < 3 else (2, g - 3)
        k0 = (-2.0 * WS * W[:, cidx * 9 + kh0 * 3 + kw]).T  # [64(c), 128(o)]
        k1 = np.zeros_like(k0)
        if g < 3:
            k1 = (-2.0 * WS * W[:, cidx * 9 + 3 + kw]).T
        if SWI:
            # hw stream: [k0[127], k1[127], k0[126], k1[126], ..., k1[0]]
            s = np.arange(256)
            col = 127 - s // 2
            stream = np.where(s % 2 == 0, k0[:, col], k1[:, col])  # [64, 256]
            lwf[0:64, g] = stream.reshape(64, 2, 128)
        else:
            lwf[0:64, g, 0, :] = k0
            lwf[0:64, g, 1, :] = k1
    lwf[64:128] = lwf[0:64]
    lwo = np.full((128, 128), 4.0, np.float32)
    w2 = (W * W).sum(axis=1).astype(np.float32).reshape(128, 1)
    return (
        lwf.astype(ml_dtypes.float8_e4m3),
        lwo.astype(ml_dtypes.bfloat16),
        w2,
    )


def get_program():
    global _PROGRAM
    if _PROGRAM is None:
        _PROGRAM = _build_program()
    return _PROGRAM


def make_in_maps(x, W):
    x = np.asarray(x, np.float32)
    xpad = np.zeros((N, C, HP, WP), np.float32)
    xpad[:, :, 1 : H + 1, 1 : W_DIM + 1] = x * XS
    xpad8 = xpad.astype(ml_dtypes.float8_e4m3)
    lwf, lwo, w2 = _host_weights(W)
    return [
        {
            "xs8": xpad8[i * NL : (i + 1) * NL],
            "lwf": lwf,
            "lwo": lwo,
            "w2": w2,
        }
        for i in range(NCORES)
    ]


def kernel(x, W):
    nc = get_program()
    in_maps = make_in_maps(x, W)
    res = run_bass_kernel_spmd(nc, in_maps, list(range(NCORES)))
    outs = [res.results[i]["out"] for i in range(NCORES)]
    return np.concatenate(outs, axis=0)


# revision 24
# speedup vs baseline: 1.0395x; 1.0395x over previous
"""Trainium2 Bass kernel for Conv2D_DT (distance-transform conv).

d(n,o,h,w) = || patch(n,:,h,w) - W[o,:] ||_2  with 3x3/pad1 im2col patches.

Strategy (8 NeuronCores, data-parallel over batch; fp8 DoubleRow taps):
  - 4 images per core as 2 pairs: image A on SBUF partitions 0-63,
    image B on 64-127 (channels = partition dim). K=64 matmuls for the
    two images land on PE row-groups (0,0)/(64,0) and run concurrently.
  - x is shipped ONCE as fp8e4 (value 8*x), padded to [C, 58, 64] so an
    8-row x 64-col chunk is contiguous (512 f32 = exactly one PSUM bank)
    and DoubleRow k-tile strides are 16B-aligned.
  - d2 = ||p||^2 + ||w||^2 - 2 p.w in PSUM, scale 256 = (8)*(32):
      * 6 fp8 DoubleRow matmuls per chunk: row pairs (0,kw)+(1,kw) and
        (2,kw)+zero-phantom, lhsT = fp8(-64*W_tap): 2 taps per pass.
      * 1 bf16 matmul with lhsT = 4.0 over b = 3x3 box sum of (8x)^2,
        computing the whole 256*||p||^2 term via the K=64 contraction.
  - box sums run as bf16 scalar_tensor_tensor on flat [128, R*64] views
    (packed 2-byte operands -> 4x DVE mode); squares (fp8 in, 1x) are
    split between GpSimd and DVE to keep both below the DMA roofline.
  - epilogue: ScalarE  out = Sqrt(psum/256 + w2[o]), two chunks batched
    into one [128,16,56] tile per image -> 3.5KB/partition output DMAs.
  - chunk b-matmul/epilogue deferred 3 chunks (8 PSUM banks) so the PE
    queue front is taps only; preprocessing never stalls the PE.
"""

import sys

_REPO = "/opt/trn_rl_repo"
if _REPO not in sys.path:
    sys.path.insert(0, _REPO)

import bass_rust
import ml_dtypes
import numpy as np

import concourse.bass as bass  # noqa: F401
import concourse.mybir as mybir
import concourse.tile as tile
from concourse import bacc
from concourse.bass_utils import run_bass_kernel_spmd

# Problem geometry (hardcoded per harness contract).
N, C, H, W_DIM, O = 32, 64, 56, 56, 128
NCORES = 8
NL = N // NCORES  # images per core
NPAIR = NL // 2  # image pairs per core
HP = 58  # zero-padded row count
WP = 64  # padded row width (56 data + 1+1 zero pad + 6 zero tail)
RCH = 8  # output rows per PSUM chunk ([128, 8, 64] f32 = one bank)
NCH = H // RCH  # 7 chunks per image
NGRP = 6  # DoubleRow tap groups per chunk
DELAY = 1  # double-chunks between taps and epilogue (2x 4 banks in flight)
XS, WS = 8.0, 32.0  # fp8 pre-scales for x and -2W (epilogue undoes 1/256)
SWI = False  # DoubleRowSwInterleave: no measured benefit over DoubleRow

F32 = mybir.dt.float32
BF16 = mybir.dt.bfloat16
FP8 = mybir.dt.float8e4

_PROGRAM = None


def _flat(t, row, col, count):
    """[128, count] stride-1 view of tile t starting at (row, col)."""
    sl = t[0:128, row : row + 1, col : col + 1]
    sl.ap = bass_rust.VecI64Pair([list(sl.ap[0]), [1, count]])
    return sl


def _dr_rhs(t, hsl, row, col):
    """DoubleRow ifmap view [64, 2, 8, 64]: k-tile stride WP (next row),
    8 output rows x 64 cols contiguous."""
    sl = t[hsl, row : row + 1, col : col + 1]
    sl.ap = bass_rust.VecI64Pair([list(sl.ap[0]), [WP, 2], [WP, RCH], [1, WP]])
    return sl


def _build_program():
    nc = bacc.Bacc(
        "TRN2",
        target_bir_lowering=False,
        debug=False,
        enable_asserts=False,
        num_devices=NCORES,
    )
    xs8 = nc.dram_tensor("xs8", [NL, C, HP, WP], FP8, kind="ExternalInput")
    lwf = nc.dram_tensor("lwf", [128, NGRP, 2, 128], FP8, kind="ExternalInput")
    lwo = nc.dram_tensor("lwo", [128, 128], BF16, kind="ExternalInput")
    w2 = nc.dram_tensor("w2", [128, 1], F32, kind="ExternalInput")
    out = nc.dram_tensor("out", [NL, O, H, W_DIM], F32, kind="ExternalOutput")

    DR = (
        mybir.MatmulPerfMode.DoubleRowSwInterleave
        if SWI
        else mybir.MatmulPerfMode.DoubleRow
    )
    MUL = mybir.AluOpType.mult
    ADD = mybir.AluOpType.add

    with tile.TileContext(nc) as tc:
        with (
            tc.tile_pool(name="const", bufs=1) as cpool,
            tc.tile_pool(name="xin", bufs=4) as xpool,
            tc.tile_pool(name="imgs", bufs=2) as ipool,
            tc.tile_pool(name="outs", bufs=4) as opool,
            tc.tile_pool(name="psum", bufs=4, space="PSUM") as ppool,
        ):
            lwft = cpool.tile([128, NGRP, 2, 128], FP8)
            nc.sync.dma_start(out=lwft[:], in_=lwf[:, :, :, :])
            lwot = cpool.tile([128, 128], BF16)
            nc.sync.dma_start(out=lwot[:], in_=lwo[:, :])
            w2t = cpool.tile([128, 1], F32)
            nc.sync.dma_start(out=w2t[:], in_=w2[:, :])

            # halves: (first padded row, DMA'd rows, chunks); tiles get two
            # extra zeroed rows so shifted/wrapping views never leave the
            # tile or read stale SBUF.
            HALVES = ((0, 35, (0, 1, 2, 3)), (32, 26, (4, 5, 6)))

            # Front-load every input DMA (and spare-row memsets) so the DMA
            # queue head is pure input while compute ramps.
            pair_x = []
            for p in range(NPAIR):
                na, nb = 2 * p, 2 * p + 1
                xh = []
                for r0, RD, _chs in HALVES:
                    x8t = xpool.tile([128, RD + 2, WP], FP8, tag="x8")
                    # one descriptor for both images: partition = img*64+ch;
                    # issued on the ACT hwdge queue so descriptor generation
                    # overlaps the sync queue's const/output DMAs.
                    nc.scalar.dma_start(
                        out=x8t[:, 0:RD, :],
                        in_=xs8[na : nb + 1, :, r0 : r0 + RD, :],
                    )
                    nc.vector.memset(x8t[:, RD : RD + 2, :], 0)
                    xh.append(x8t)
                pair_x.append(xh)

            def finish(item):
                chs2, na, nb, psa, psb, bt, r0 = item
                hh = len(chs2) * RCH
                for hsl, ps in ((slice(0, 64), psa), (slice(64, 128), psb)):
                    for si, ch in enumerate(chs2):
                        lb = ch * RCH - r0
                        # channel-sum of the 3x3 box term: one bf16 matmul
                        nc.tensor.matmul(
                            ps[:, si * RCH : (si + 1) * RCH, :],
                            lwot[hsl, :],
                            bt[hsl, lb : lb + RCH, :],
                            start=False,
                            stop=True,
                        )
                h0 = chs2[0] * RCH
                for ps, n_img in ((psa, na), (psb, nb)):
                    ot = opool.tile([128, 16, W_DIM], F32, tag="ot", name="ot")
                    nc.scalar.activation(
                        out=ot[:, 0:hh, :],
                        in_=ps[:, 0:hh, 0:W_DIM],
                        func=mybir.ActivationFunctionType.Sqrt,
                        bias=w2t[:],
                        scale=1.0 / 256.0,
                    )
                    nc.sync.dma_start(
                        out=out[n_img, :, h0 : h0 + hh, :], in_=ot[:, 0:hh, :]
                    )

            pending = []
            for p in range(NPAIR):
                na, nb = 2 * p, 2 * p + 1
                halves = []
                for (r0, RD, chs), x8t in zip(HALVES, pair_x[p]):
                    nr = RD + 1  # sq rows (incl one zeroed spare)
                    sqt = ipool.tile([128, nr, WP], BF16, tag="sq")
                    sqs = ipool.tile([128, RD, WP], BF16, tag="sqs")
                    ut = ipool.tile([128, RD, WP], BF16, tag="u")
                    ttt = ipool.tile([128, RD, WP], BF16, tag="tt")
                    vt = ipool.tile([128, RD - 1, WP], BF16, tag="v")
                    bt = ipool.tile([128, RD - 2, WP], BF16, tag="b")
                    # The very first half gates the PE pipeline: emit a short
                    # leading row-slice first so the first double-chunk's box
                    # term is ready a few us earlier, then the remainder.
                    # Ranges per segment: (square, shifted h-sums, v, b) —
                    # each op lags its input by the rows it peeks ahead.
                    if p == 0 and r0 == 0:
                        segs = (
                            ((0, 22), (0, 21), (0, 20), (0, 19)),
                            ((22, nr), (21, RD), (20, RD - 1), (19, RD - 2)),
                        )
                    else:
                        segs = (((0, nr), (0, RD), (0, RD - 1), (0, RD - 2)),)
                    for (q0, q1), (s0, s1), (v0, v1), (b0, b1) in segs:
                        nf = (q1 - q0) * WP
                        ns = (s1 - s0) * WP
                        nv = (v1 - v0) * WP
                        nb2 = (b1 - b0) * WP
                        # squares (fp8 in, 1x anywhere): first half on ScalarE
                        # (its sqrt backlog is empty early), rest on GpSimd.
                        if r0 == 0:
                            nc.scalar.activation(
                                out=_flat(sqt, q0, 0, nf),
                                in_=_flat(x8t, q0, 0, nf),
                                func=mybir.ActivationFunctionType.Square,
                            )
                        else:
                            nc.gpsimd.tensor_mul(
                                _flat(sqt, q0, 0, nf),
                                _flat(x8t, q0, 0, nf),
                                _flat(x8t, q0, 0, nf),
                            )
                        # +1-elem shifted replica via single-src copy (2x_2p
                        # needs no alignment); keeps adds 4B-aligned for 2x_1p.
                        nc.vector.tensor_copy(
                            _flat(sqs, s0, 0, ns), _flat(sqt, s0, 1, ns)
                        )
                        nc.vector.tensor_add(
                            _flat(ut, s0, 0, ns),
                            _flat(sqt, s0, 0, ns),
                            _flat(sqs, s0, 0, ns),
                        )
                        nc.vector.tensor_add(
                            _flat(ttt, s0, 0, ns),
                            _flat(ut, s0, 0, ns),
                            _flat(sqt, s0, 2, ns),
                        )
                        # vertical 3x1 box sum: v on DVE; b split DVE/GpSimd
                        nc.vector.tensor_add(
                            _flat(vt, v0, 0, nv),
                            _flat(ttt, v0, 0, nv),
                            _flat(ttt, v0 + 1, 0, nv),
                        )
                        b_eng = nc.vector if r0 == 0 else nc.gpsimd
                        b_eng.tensor_add(
                            _flat(bt, b0, 0, nb2),
                            _flat(vt, b0, 0, nb2),
                            _flat(ttt, b0 + 2, 0, nb2),
                        )
                    halves.append((r0, chs, x8t, bt))

                for r0, chs, x8t, bt in halves:
                    for chs2 in (chs[0:2], chs[2:4]):
                        if not chs2:
                            continue
                        psa = ppool.tile([128, 16, WP], F32, tag="ps")
                        psb = ppool.tile([128, 16, WP], F32, tag="ps")
                        for si, ch in enumerate(chs2):
                            lh = ch * RCH - r0  # chunk's first row in half
                            for g in range(NGRP):
                                kh0, kw = (0, g) if g < 3 else (2, g - 3)
                                st = g == 0
                                for hsl, ps in (
                                    (slice(0, 64), psa),
                                    (slice(64, 128), psb),
                                ):
                                    nc.tensor.matmul(
                                        ps[:, si * RCH : (si + 1) * RCH, :],
                                        lwft[hsl, g],
                                        _dr_rhs(x8t, hsl, lh + kh0, kw),
                                        start=st,
                                        stop=False,
                                        perf_mode=DR,
                                    )
                        pending.append((chs2, na, nb, psa, psb, bt, r0))
                        if len(pending) > DELAY:
                            finish(pending.pop(0))
            for item in pending:
                finish(item)
    nc.compile()
    return nc


def _host_weights(W):
    """fp8 DoubleRow lhsT [128, 6, 2, 128], bf16 4.0 matrix, f32 w2."""
    W = np.asarray(W, np.float32)
    cidx = np.arange(C)
    lwf = np.zeros((128, NGRP, 2, 128), np.float32)
    for g in range(NGRP):
        kh0, kw = (0, g) if g < 3 else (2, g - 3)
        k0 = (-2.0 * WS * W[:, cidx * 9 + kh0 * 3 + kw]).T  # [64(c), 128(o)]
        k1 = np.zeros_like(k0)
        if g < 3:
            k1 = (-2.0 * WS * W[:, cidx * 9 + 3 + kw]).T
        if SWI:
            # hw stream: [k0[127], k1[127], k0[126], k1[126], ..., k1[0]]
            s = np.arange(256)
            col = 127 - s // 2
            stream = np.where(s % 2 == 0, k0[:, col], k1[:, col])  # [64, 256]
            lwf[0:64, g] = stream.reshape(64, 2, 128)
        else:
            lwf[0:64, g, 0, :] = k0
            lwf[0:64, g, 1, :] = k1
    lwf[64:128] = lwf[0:64]
    lwo = np.full((128, 128), 4.0, np.float32)
    w2 = (W * W).sum(axis=1).astype(np.float32).reshape(128, 1)
    return (
        lwf.astype(ml_dtypes.float8_e4m3),
        lwo.astype(ml_dtypes.bfloat16),
        w2,
    )


def get_program():
    global _PROGRAM
    if _PROGRAM is None:
        _PROGRAM = _build_program()
    return _PROGRAM


def make_in_maps(x, W):
    x = np.asarray(x, np.float32)
    xpad = np.zeros((N, C, HP, WP), np.float32)
    xpad[:, :, 1 : H + 1, 1 : W_DIM + 1] = x * XS
    xpad8 = xpad.astype(ml_dtypes.float8_e4m3)
    lwf, lwo, w2 = _host_weights(W)
    return [
        {
            "xs8": xpad8[i * NL : (i + 1) * NL],
            "lwf": lwf,
            "lwo": lwo,
            "w2": w2,
        }
        for i in range(NCORES)
    ]


def kernel(x, W):
    nc = get_program()
    in_maps = make_in_maps(x, W)
    res = run_bass_kernel_spmd(nc, in_maps, list(range(NCORES)))
    outs = [res.results[i]["out"] for i in range(NCORES)]
    return np.concatenate(outs, axis=0)
